# revision 134
# baseline (speedup 1.0000x reference)
"""Trainium2 Bass kernel for NewExpressionAttentionLayer (sparse gated attention).

Math (per batch b):
  X = concat(gene, expr); Q = X @ (W_fused W_Q scale); K = X @ (W_fused W_K)
  V = expr @ W_V                      (weight folding done on host, fp32)
  t = (Q K^T) * M                     (M = gate)
  p = exp(t)                          (softmax without max-subtraction; |t| <~ 6)
  pm = p * M
  A_bar = pm / sum_k(pm)              (softmax Z cancels; EPS is O(1e-8) rel -> dropped)
  out = (A_bar @ V) @ W_O + b_O       (b_O added on host)

Sharding: 8 cores = 4 batches x 2 head-halves (tensor parallel on heads).
Each core projects Q/K/V for its 4 heads over all 2048 positions and runs
attention for all queries; the host sums the two partial W_O projections.
Head-splitting halves the projection matmul work vs query-splitting (K/V
would be duplicated across the pair) at the cost of shipping the full
[S,S] gate to each core.

Device layout is feature-major: activations [feat, seq] so the PE (contracting
along partitions) needs no on-device transposes. Scores are computed transposed
(scoresT[k, q] = K^T_h.T @ Q^T_h) into fp32 PSUM pair-tiles [128,2,512].
Everything off the PE path is fp16 (2-byte dtype gets DVE 2x mode; fp16
matmuls run 1 row/cycle at any free size).

The attention inner loop is software-pipelined over stages (qc, h, g) where
qc = query chunk of 512, h = local head, g = 4 kt tiles. Front half emits
scores + gate-mul u = scoresT*M + exp; back half (LAG stages later) emits
pm = e*M + 4 AV accumulate matmuls. The gate-mul must read fp32 PSUM, which
only DVE can touch: a tunable fraction is "assisted" (Activation copies the
pair to fp16 SBUF, DVE multiplies in 2x mode) and pm = e*M splits between
DVE (2x) and GpSimd. Normalization, inverse-scale and the W_O projection are
scheduled as side-emissions inside the stage stream so nothing serializes at
chunk boundaries except the final tail.
"""

import sys

sys.path.insert(0, "/opt/trn_rl_repo")

import numpy as np

B, S, D = 4, 2048, 512
H, HD = 8, 64
HL = H // 2          # local heads per core
DL = HL * HD         # 256 local head dims
KT = S // 128        # 16 k partition tiles
QC_W = 512           # query columns per score matmul (PSUM bank limit)
N_QC = S // QC_W     # 4

_PROG = None


def _build_program():
    from contextlib import ExitStack

    from concourse import bacc, mybir
    import concourse.tile as tile

    f32 = mybir.dt.float32
    f16 = mybir.dt.float16
    Exp = mybir.ActivationFunctionType.Exp
    Copy = mybir.ActivationFunctionType.Copy
    MUL = mybir.AluOpType.mult

    nc = bacc.Bacc("TRN2", target_bir_lowering=False, debug=False, num_devices=8)

    XT = nc.dram_tensor("XT", [2 * D, S], f16, kind="ExternalInput").ap()
    MT = nc.dram_tensor("MT", [S, S], f16, kind="ExternalInput").ap()
    WXQ = nc.dram_tensor("WXQ", [2 * D, DL], f16, kind="ExternalInput").ap()
    WXK = nc.dram_tensor("WXK", [2 * D, DL], f16, kind="ExternalInput").ap()
    WV = nc.dram_tensor("WV", [D, DL], f16, kind="ExternalInput").ap()
    WO = nc.dram_tensor("WO", [DL, D], f16, kind="ExternalInput").ap()
    OUT = nc.dram_tensor("OUT", [S, D], f32, kind="ExternalOutput").ap()

    with tile.TileContext(nc) as tc, ExitStack() as _ctx:
            _pool = lambda *a, **k: _ctx.enter_context(tc.tile_pool(*a, **k))
            misc = _pool(name="misc", bufs=1)
            kqv = _pool(name="kqv", bufs=1)
            mtp = _pool(name="mtp", bufs=2)
            up = _pool(name="up", bufs=3)
            ep = _pool(name="ep", bufs=5)
            pmp = _pool(name="pmp", bufs=7)
            ovtp = _pool(name="ovtp", bufs=2)
            ivtp = _pool(name="ivtp", bufs=2)
            rvp = _pool(name="rvp", bufs=2)
            s16p = _pool(name="s16p", bufs=2)
            finp = _pool(name="finp", bufs=2)
            psS = _pool(name="psS", bufs=3, space="PSUM")   # [128,2,512] pairs
            psA = _pool(name="psA", bufs=2, space="PSUM")   # av accumulators

            ones64 = misc.tile([128, 64], f16)
            nc.vector.memset(ones64, 1.0)
            wo2_sb = misc.tile([64, HL, D], f16)

            kt_sb = kqv.tile([128, 2, S], f16)      # K^T [d_local, s]
            qt_sb = kqv.tile([128, 2, S], f16)      # Q^T [d_local, q]
            v_sb = kqv.tile([128, KT, HL, HD + 1], f16)  # V + ones col
            nc.vector.memset(v_sb[:, :, :, HD : HD + 1], 1.0)

            mt_r = MT.rearrange("(t p) q -> p t q", p=128)
            mt_sb = [None] * N_QC

            def emit_mt_dma(qc):
                mt_c = mtp.tile([128, KT, QC_W], f16, tag="mt", name=f"mt{qc}")
                mt_sb[qc] = mt_c
                for q4 in range(4):
                    nc.sync.dma_start(
                        out=mt_c[:, q4 * 4 : (q4 + 1) * 4, :],
                        in_=mt_r[:, q4 * 4 : (q4 + 1) * 4, qc * QC_W : (qc + 1) * QC_W],
                    )

            # ------------- attention stage emitters (software pipelined) -------------
            ovt_sb = [None] * N_QC
            iv_t = [None] * N_QC
            ps_av = {}

            def front(qc, h, g):
                qsl = slice(qc * QC_W, (qc + 1) * QC_W)
                mt = mt_sb[qc]
                hoff = (h % 2) * 64
                ht = h // 2
                if (h, g) == (0, 0):
                    ovt_sb[qc] = ovtp.tile(
                        [HD + 1, HL, QC_W], f16, tag="ovt", name=f"ovt{qc}"
                    )
                    iv_t[qc] = ivtp.tile(
                        [HD + 1, HL, QC_W], f16, tag="ivt", name=f"ivt{qc}"
                    )
                if g == 0:
                    ps_av[(qc, h)] = psA.tile(
                        [HD + 1, QC_W], f32, tag="a", name=f"psav{qc}_{h}"
                    )
                u = up.tile([128, 4, QC_W], f16, tag="u")
                for jp in range(2):
                    ps_s = psS.tile([128, 2, QC_W], f32, tag="s2", name=f"pss{h}_{g}_{jp}")
                    for j2 in range(2):
                        kt = 4 * g + 2 * jp + j2
                        nc.tensor.matmul(
                            ps_s[:, j2, :],
                            kt_sb[hoff : hoff + 64, ht, kt * 128 : (kt + 1) * 128],
                            qt_sb[hoff : hoff + 64, ht, qsl],
                            start=True, stop=True,
                        )
                    # gate-multiply reads fp32 PSUM: GpSimd can't touch PSUM,
                    # so this is DVE-only (the dominant DVE cost). For a
                    # fraction of pairs, Activation (which has slack) copies
                    # the pair to fp16 SBUF so DVE runs it in 2x mode.
                    p = pair_ctr[0]
                    pair_ctr[0] += 1
                    msl = mt[:, 4 * g + 2 * jp : 4 * g + 2 * jp + 2, :]
                    usl = u[:, 2 * jp : 2 * jp + 2, :]
                    if p % 2 == 0 and ((p // 2) * MUL1_ASSIST) % 64 < MUL1_ASSIST:
                        s16 = s16p.tile([128, 2, QC_W], f16, tag="s16")
                        nc.scalar.activation(s16, ps_s, Copy)
                        # once in fp16 SBUF the multiply can also run on
                        # GpSimd - send a few there to shave the DVE wall
                        a = asst_ctr[0]
                        asst_ctr[0] += 1
                        eng = nc.vector
                        eng.tensor_tensor(usl, s16, msl, MUL)
                    else:
                        nc.vector.tensor_tensor(usl, ps_s, msl, MUL)
                e = ep.tile([128, 4, QC_W], f16, tag="e")
                nc.scalar.activation(e, u, Exp)
                return e

            mul2_ctr = [0]
            pair_ctr = [0]
            asst_ctr = [0]
            MUL2_POOL = 66   # of 128 fp16 halves, rest on DVE (2x mode)
            MUL1_ASSIST = 24  # of 128 gate-mul pairs get the Act-assisted path

            def back_elem(qc, h, g, e):
                mt = mt_sb[qc]
                pm = pmp.tile([128, 4, QC_W], f16, tag="pm")
                # pm = e * M in all-SBUF fp16 halves, split DVE (2x mode) /
                # GpSimd to balance (Pool is ~3.6x slower on fp16)
                for jp in range(2):
                    i = mul2_ctr[0]
                    mul2_ctr[0] += 1
                    pool_take = (i % 2) == 1 or (i % 32) == 0
                    if (qc, h) == (N_QC - 1, HL - 1):
                        pool_take = False   # tail stages: DVE is idle there
                    eng = nc.gpsimd if pool_take else nc.vector
                    eng.tensor_tensor(
                        pm[:, 2 * jp : 2 * jp + 2, :],
                        e[:, 2 * jp : 2 * jp + 2, :],
                        mt[:, 4 * g + 2 * jp : 4 * g + 2 * jp + 2, :], MUL,
                    )
                return pm

            def back_av(qc, h, g, pm):
                for j in range(4):
                    kt = 4 * g + j
                    nc.tensor.matmul(
                        ps_av[(qc, h)], v_sb[:, kt, h, :], pm[:, j, :],
                        start=(kt == 0), stop=(kt == KT - 1),
                    )
                if g == 3:
                    nc.scalar.activation(ovt_sb[qc][:, h, :], ps_av[(qc, h)], Copy)

            def emit_norm(qc, hs):
                """1/norm (fp16) for heads hs in one DVE reciprocal."""
                ovt = ovt_sb[qc]
                h0, h1 = hs[0], hs[-1] + 1
                with nc.allow_low_precision(reason="1/norm fp16; norm ~1e3"):
                    nc.vector.reciprocal(
                        iv_t[qc][HD : HD + 1, h0:h1, :], ovt[HD : HD + 1, h0:h1, :]
                    )

            def emit_invmul(qc, h, eng=None):
                # broadcast 1/norm across 64 partitions via PE, land fp16 in
                # SBUF, scale the head output in place on GpSimd (or DVE in
                # the tail where it's idle)
                ps_r = psS.tile([128, 2, QC_W], f32, tag="s2", name=f"psr{qc}_{h}")
                nc.tensor.matmul(
                    ps_r[0:HD, 0, :], ones64[HD : HD + 1, 0:HD],
                    iv_t[qc][HD : HD + 1, h, :], start=True, stop=True,
                )
                rv = rvp.tile([HD, QC_W], f16, tag="rv")
                nc.scalar.activation(rv, ps_r[0:HD, 0, :], Copy)
                (eng or nc.gpsimd).tensor_tensor(
                    ovt_sb[qc][0:HD, h, :], ovt_sb[qc][0:HD, h, :], rv, MUL
                )

            def emit_o_tile(qc, qtl):
                ovn = ovt_sb[qc][0:HD]
                qt_g = qc * (QC_W // 128) + qtl
                ps_o = psS.tile([128, 2, D], f32, tag="s2", name=f"pso{qc}_{qtl}")
                for h in range(HL):
                    nc.tensor.matmul(
                        ps_o[:, 0, :], ovn[:, h, qtl * 128 : (qtl + 1) * 128],
                        wo2_sb[:, h, :], start=(h == 0), stop=(h == HL - 1),
                    )
                fin = finp.tile([128, D], f32, tag="fin")
                nc.scalar.activation(fin, ps_o[:, 0, :], Copy)
                nc.sync.dma_start(
                    out=OUT[qt_g * 128 : (qt_g + 1) * 128, :], in_=fin
                )

            LAG = 3
            queue = []
            push_ctr = [0]

            # side-emissions keyed by completed-push index. Push order:
            # proj chunks c=0..3 interleave (0,0,c),(0,1,c) -> pushes 0..7;
            # then qc0 h2..3 -> 8..15; qc1 h0..3 -> 16..31; qc2 -> 32..47;
            # qc3 -> 48..63. back(i) drains at push i+LAG.
            extras = {}

            def _extra(i, fn):
                extras.setdefault(i, []).append(fn)

            _extra(14, lambda: emit_mt_dma(2))
            _extra(35, lambda: emit_mt_dma(3))
            # back_av (which emits the ovt copies) runs ~4 pushes behind the
            # front: every norm must be EMITTED after its heads' ovt copies
            # (Tile deps follow emission order), and >=1 push before its
            # first invmul so the PE broadcast doesn't head-of-line block.
            _extra(22, lambda: emit_norm(0, [0, 1, 2, 3]))
            for k in range(4):
                _extra(23 + k, lambda k=k: emit_invmul(0, k))
            for k in range(4):
                _extra(27 + 2 * k, lambda k=k: emit_o_tile(0, k))
            _extra(38, lambda: emit_norm(1, [0, 1, 2, 3]))
            for k in range(4):
                _extra(39 + k, lambda k=k: emit_invmul(1, k))
            for k in range(4):
                _extra(43 + 2 * k, lambda k=k: emit_o_tile(1, k))
            _extra(54, lambda: emit_norm(2, [0, 1, 2, 3]))
            for k in range(4):
                _extra(55 + k, lambda k=k: emit_invmul(2, k))
            _extra(59, lambda: emit_o_tile(2, 0))
            _extra(61, lambda: emit_o_tile(2, 1))
            _extra(62, lambda: emit_o_tile(2, 2))
            _extra(63, lambda: emit_o_tile(2, 3))
            _extra(58, lambda: emit_norm(3, [0]))
            _extra(61, lambda: emit_invmul(3, 0))
            _extra(62, lambda: emit_norm(3, [1]))
            _extra(63, lambda: emit_invmul(3, 1))

            av_queue = []
            # during the PE-bound projection window the AV matmuls of early
            # stages are deferred (deep av queue); the attention window (where
            # PE has slack) absorbs them at 2 per push
            av_lag = [99]

            def push(qc, h, g):
                queue.append((qc, h, g, front(qc, h, g)))
                if len(queue) > LAG:
                    qh = queue.pop(0)
                    av_queue.append((qh[0], qh[1], qh[2], back_elem(*qh)))
                burst = 2
                while len(av_queue) > av_lag[0] and burst:
                    back_av(*av_queue.pop(0))
                    burst -= 1
                for fn in extras.get(push_ctr[0], ()):
                    fn()
                push_ctr[0] += 1

            def drain():
                while queue:
                    qh = queue.pop(0)
                    av_queue.append((qh[0], qh[1], qh[2], back_elem(*qh)))
                    while len(av_queue) > 1:
                        back_av(*av_queue.pop(0))
                while av_queue:
                    back_av(*av_queue.pop(0))

            # ---------------- projection phase (+ early attention stages) ----------
            _proj_ctx = ExitStack()
            if True:
                projw = _proj_ctx.enter_context(tc.tile_pool(name="projw", bufs=1))
                xtp = _proj_ctx.enter_context(tc.tile_pool(name="xtp", bufs=2))
                # DMA order: the first K accumulation steps need only the
                # first halves of wxk and xt chunk 0 - split those DMAs so
                # the PE starts earlier
                wxk_sb = projw.tile([128, 8, DL], f16)
                wxk_r = WXK.rearrange("(t p) n -> p t n", p=128)
                nc.sync.dma_start(out=wxk_sb[:, 0:4, :], in_=wxk_r[:, 0:4, :])
                wxq_sb = projw.tile([128, 8, DL], f16)
                wv_sb = projw.tile([128, 4, DL], f16)

                xt_r = XT.rearrange("(t p) s -> p t s", p=128)
                for c in range(4):
                    csl = slice(c * 512, (c + 1) * 512)
                    xt_c = xtp.tile([128, 8, 512], f16, tag="xt")
                    if c == 0:
                        nc.sync.dma_start(out=xt_c[:, 0:4, :], in_=xt_r[:, 0:4, csl])
                        nc.sync.dma_start(out=wxk_sb[:, 4:8, :], in_=wxk_r[:, 4:8, :])
                        nc.sync.dma_start(out=xt_c[:, 4:8, :], in_=xt_r[:, 4:8, csl])
                    else:
                        nc.sync.dma_start(out=xt_c, in_=xt_r[:, :, csl])
                    if c == 0:
                        nc.sync.dma_start(out=wxq_sb, in_=WXQ.rearrange("(t p) n -> p t n", p=128))
                        nc.sync.dma_start(out=wv_sb, in_=WV.rearrange("(t p) n -> p t n", p=128))
                        emit_mt_dma(0)
                    elif c == 1:
                        emit_mt_dma(1)
                        nc.sync.dma_start(
                            out=wo2_sb, in_=WO.rearrange("(h d) n -> d h n", d=HD)
                        )

                    # K^T and Q^T local-head chunks: one [128,2,512] pair each
                    for w_sb, dst in (
                        (wxk_sb, kt_sb[:, :, csl]),
                        (wxq_sb, qt_sb[:, :, csl]),
                    ):
                        ps = psS.tile([128, 2, 512], f32, tag="s2")
                        for j in range(2):
                            for t in range(8):
                                nc.tensor.matmul(
                                    ps[:, j, :], w_sb[:, t, j * 128 : (j + 1) * 128],
                                    xt_c[:, t, :], start=(t == 0), stop=(t == 7),
                                )
                            nc.scalar.activation(dst[:, j, :], ps[:, j, :], Copy)

                    for sp in range(2):         # st pairs
                        ps = psS.tile([128, 2, 512], f32, tag="s2")
                        for j in range(2):
                            st = 2 * sp + j
                            sidx = c * 4 + st
                            for dt in range(4):
                                nc.tensor.matmul(
                                    ps[:, j, 0:DL],
                                    xt_c[:, 4 + dt, st * 128 : (st + 1) * 128],
                                    wv_sb[:, dt, :], start=(dt == 0), stop=(dt == 3),
                                )
                            nc.scalar.activation(
                                v_sb[:, sidx, :, 0:HD],
                                ps[:, j, 0:DL].rearrange("p (h d) -> p h d", h=HL),
                                Copy,
                            )

                    # stage (qc, h, g) needs K/V chunks <= g and Q chunk qc:
                    # start qc0's h0/h1 early
                    push(0, 0, c)
                    push(0, 1, c)
            _proj_ctx.close()
            av_lag[0] = 3

            # ---------------- attention phase ----------------
            for qc in range(N_QC):
                for h in range(HL):
                    if qc == 0 and h < 2:
                        continue    # emitted during projections
                    for g in range(4):
                        push(qc, h, g)
            drain()
            emit_norm(3, [2])
            emit_invmul(3, 2, eng=nc.vector)
            emit_norm(3, [3])
            emit_invmul(3, 3, eng=nc.vector)
            for qtl in range(QC_W // 128):
                emit_o_tile(3, qtl)

    nc.compile()
    return nc


def _get_prog():
    global _PROG
    if _PROG is None:
        _PROG = _build_program()
    return _PROG


def _make_in_maps(inputs):
    f = lambda k: np.asarray(inputs[k], dtype=np.float32)
    gene, expr, M = f("gene_emb"), f("expr_emb"), f("M")
    W_fused = f("W_fused")
    W_Q, W_K, W_V, W_O = f("W_Q"), f("W_K"), f("W_V"), f("W_O")

    scale = np.float32(HD ** -0.5)
    WXQ = (W_fused @ W_Q) * scale
    WXK = W_fused @ W_K

    in_maps = []
    for c in range(8):
        b, hh = c // 2, c % 2
        dsl = slice(hh * DL, (hh + 1) * DL)
        xt = np.concatenate([gene[b], expr[b]], axis=1).T  # [1024, 2048]
        mt = M[b].T                                        # [2048, 2048]
        in_maps.append(
            dict(
                XT=np.ascontiguousarray(xt, dtype=np.float16),
                MT=np.ascontiguousarray(mt, dtype=np.float16),
                WXQ=np.ascontiguousarray(WXQ[:, dsl], dtype=np.float16),
                WXK=np.ascontiguousarray(WXK[:, dsl], dtype=np.float16),
                WV=np.ascontiguousarray(W_V[:, dsl], dtype=np.float16),
                WO=np.ascontiguousarray(W_O[dsl, :], dtype=np.float16),
            )
        )
    return in_maps


def kernel(**inputs) -> np.ndarray:
    from concourse.bass_utils import run_bass_kernel_spmd

    nc = _get_prog()
    in_maps = _make_in_maps(inputs)
    res = run_bass_kernel_spmd(nc, in_maps, core_ids=list(range(8)))

    b_O = np.asarray(inputs["b_O"], dtype=np.float32)
    out = np.empty((B, S, D), dtype=np.float32)
    for b in range(B):
        out[b] = res.results[2 * b]["OUT"] + res.results[2 * b + 1]["OUT"] + b_O[None, :]
    return out


# revision 135
# speedup vs baseline: 1.0038x; 1.0038x over previous
"""Trainium2 Bass kernel for NewExpressionAttentionLayer (sparse gated attention).

Math (per batch b):
  X = concat(gene, expr); Q = X @ (W_fused W_Q scale); K = X @ (W_fused W_K)
  V = expr @ W_V                      (weight folding done on host, fp32)
  t = (Q K^T) * M                     (M = gate)
  p = exp(t)                          (softmax without max-subtraction; |t| <~ 6)
  pm = p * M
  A_bar = pm / sum_k(pm)              (softmax Z cancels; EPS is O(1e-8) rel -> dropped)
  out = (A_bar @ V) @ W_O + b_O       (b_O added on host)

Sharding: 8 cores = 4 batches x 2 head-halves (tensor parallel on heads).
Each core projects Q/K/V for its 4 heads over all 2048 positions and runs
attention for all queries; the host sums the two partial W_O projections.
Head-splitting halves the projection matmul work vs query-splitting (K/V
would be duplicated across the pair) at the cost of shipping the full
[S,S] gate to each core.

Device layout is feature-major: activations [feat, seq] so the PE (contracting
along partitions) needs no on-device transposes. Scores are computed transposed
(scoresT[k, q] = K^T_h.T @ Q^T_h) into fp32 PSUM pair-tiles [128,2,512].
Everything off the PE path is fp16 (2-byte dtype gets DVE 2x mode; fp16
matmuls run 1 row/cycle at any free size).

The attention inner loop is software-pipelined over stages (qc, h, g) where
qc = query chunk of 512, h = local head, g = 4 kt tiles. Front half emits
scores + gate-mul u = scoresT*M + exp; back half (LAG stages later) emits
pm = e*M + 4 AV accumulate matmuls. The gate-mul must read fp32 PSUM, which
only DVE can touch: a tunable fraction is "assisted" (Activation copies the
pair to fp16 SBUF, DVE multiplies in 2x mode) and pm = e*M splits between
DVE (2x) and GpSimd. Normalization, inverse-scale and the W_O projection are
scheduled as side-emissions inside the stage stream so nothing serializes at
chunk boundaries except the final tail.
"""

import sys

sys.path.insert(0, "/opt/trn_rl_repo")

import numpy as np

B, S, D = 4, 2048, 512
H, HD = 8, 64
HL = H // 2          # local heads per core
DL = HL * HD         # 256 local head dims
KT = S // 128        # 16 k partition tiles
QC_W = 512           # query columns per score matmul (PSUM bank limit)
N_QC = S // QC_W     # 4

_PROG = None


def _build_program():
    from contextlib import ExitStack

    from concourse import bacc, mybir
    import concourse.tile as tile

    f32 = mybir.dt.float32
    f16 = mybir.dt.float16
    Exp = mybir.ActivationFunctionType.Exp
    Copy = mybir.ActivationFunctionType.Copy
    MUL = mybir.AluOpType.mult

    nc = bacc.Bacc("TRN2", target_bir_lowering=False, debug=False, num_devices=8)

    XT = nc.dram_tensor("XT", [2 * D, S], f16, kind="ExternalInput").ap()
    MT = nc.dram_tensor("MT", [S, S], f16, kind="ExternalInput").ap()
    WXQ = nc.dram_tensor("WXQ", [2 * D, DL], f16, kind="ExternalInput").ap()
    WXK = nc.dram_tensor("WXK", [2 * D, DL], f16, kind="ExternalInput").ap()
    WV = nc.dram_tensor("WV", [D, DL], f16, kind="ExternalInput").ap()
    WO = nc.dram_tensor("WO", [DL, D], f16, kind="ExternalInput").ap()
    OUT = nc.dram_tensor("OUT", [S, D], f32, kind="ExternalOutput").ap()

    with tile.TileContext(nc) as tc, ExitStack() as _ctx:
            _pool = lambda *a, **k: _ctx.enter_context(tc.tile_pool(*a, **k))
            misc = _pool(name="misc", bufs=1)
            kqv = _pool(name="kqv", bufs=1)
            mtp = _pool(name="mtp", bufs=2)
            up = _pool(name="up", bufs=3)
            ep = _pool(name="ep", bufs=5)
            pmp = _pool(name="pmp", bufs=7)
            ovtp = _pool(name="ovtp", bufs=2)
            ivtp = _pool(name="ivtp", bufs=2)
            rvp = _pool(name="rvp", bufs=2)
            s16p = _pool(name="s16p", bufs=2)
            finp = _pool(name="finp", bufs=2)
            psS = _pool(name="psS", bufs=3, space="PSUM")   # [128,2,512] pairs
            psA = _pool(name="psA", bufs=2, space="PSUM")   # av accumulators

            ones64 = misc.tile([128, 64], f16)
            nc.vector.memset(ones64, 1.0)
            wo2_sb = misc.tile([64, HL, D], f16)

            kt_sb = kqv.tile([128, 2, S], f16)      # K^T [d_local, s]
            qt_sb = kqv.tile([128, 2, S], f16)      # Q^T [d_local, q]
            v_sb = kqv.tile([128, KT, HL, HD + 1], f16)  # V + ones col
            nc.vector.memset(v_sb[:, :, :, HD : HD + 1], 1.0)

            mt_r = MT.rearrange("(t p) q -> p t q", p=128)
            mt_sb = [None] * N_QC

            def emit_mt_dma(qc):
                mt_c = mtp.tile([128, KT, QC_W], f16, tag="mt", name=f"mt{qc}")
                mt_sb[qc] = mt_c
                for q4 in range(4):
                    nc.sync.dma_start(
                        out=mt_c[:, q4 * 4 : (q4 + 1) * 4, :],
                        in_=mt_r[:, q4 * 4 : (q4 + 1) * 4, qc * QC_W : (qc + 1) * QC_W],
                    )

            # ------------- attention stage emitters (software pipelined) -------------
            ovt_sb = [None] * N_QC
            iv_t = [None] * N_QC
            ps_av = {}

            def front(qc, h, g):
                qsl = slice(qc * QC_W, (qc + 1) * QC_W)
                mt = mt_sb[qc]
                hoff = (h % 2) * 64
                ht = h // 2
                if (h, g) == (0, 0):
                    ovt_sb[qc] = ovtp.tile(
                        [HD + 1, HL, QC_W], f16, tag="ovt", name=f"ovt{qc}"
                    )
                    iv_t[qc] = ivtp.tile(
                        [HD + 1, HL // 2, QC_W], f16, tag="ivt", name=f"ivt{qc}"
                    )
                if g == 0:
                    ps_av[(qc, h)] = psA.tile(
                        [HD + 1, QC_W], f32, tag="a", name=f"psav{qc}_{h}"
                    )
                u = up.tile([128, 4, QC_W], f16, tag="u")
                for jp in range(2):
                    ps_s = psS.tile([128, 2, QC_W], f32, tag="s2", name=f"pss{h}_{g}_{jp}")
                    for j2 in range(2):
                        kt = 4 * g + 2 * jp + j2
                        nc.tensor.matmul(
                            ps_s[:, j2, :],
                            kt_sb[hoff : hoff + 64, ht, kt * 128 : (kt + 1) * 128],
                            qt_sb[hoff : hoff + 64, ht, qsl],
                            start=True, stop=True,
                        )
                    # gate-multiply reads fp32 PSUM: GpSimd can't touch PSUM,
                    # so this is DVE-only (the dominant DVE cost). For a
                    # fraction of pairs, Activation (which has slack) copies
                    # the pair to fp16 SBUF so DVE runs it in 2x mode.
                    p = pair_ctr[0]
                    pair_ctr[0] += 1
                    msl = mt[:, 4 * g + 2 * jp : 4 * g + 2 * jp + 2, :]
                    usl = u[:, 2 * jp : 2 * jp + 2, :]
                    if p % 2 == 0 and ((p // 2) * MUL1_ASSIST) % 64 < MUL1_ASSIST:
                        s16 = s16p.tile([128, 2, QC_W], f16, tag="s16")
                        nc.scalar.activation(s16, ps_s, Copy)
                        # once in fp16 SBUF the multiply can also run on
                        # GpSimd - send a few there to shave the DVE wall
                        a = asst_ctr[0]
                        asst_ctr[0] += 1
                        eng = nc.vector
                        eng.tensor_tensor(usl, s16, msl, MUL)
                    else:
                        nc.vector.tensor_tensor(usl, ps_s, msl, MUL)
                e = ep.tile([128, 4, QC_W], f16, tag="e")
                nc.scalar.activation(e, u, Exp)
                return e

            mul2_ctr = [0]
            pair_ctr = [0]
            asst_ctr = [0]
            MUL2_POOL = 66   # of 128 fp16 halves, rest on DVE (2x mode)
            MUL1_ASSIST = 24  # of 128 gate-mul pairs get the Act-assisted path

            def back_elem(qc, h, g, e):
                mt = mt_sb[qc]
                pm = pmp.tile([128, 4, QC_W], f16, tag="pm")
                # pm = e * M in all-SBUF fp16 halves, split DVE (2x mode) /
                # GpSimd to balance (Pool is ~3.6x slower on fp16)
                for jp in range(2):
                    i = mul2_ctr[0]
                    mul2_ctr[0] += 1
                    pool_take = (i % 2) == 1 or (i % 32) == 0
                    if (qc, h) == (N_QC - 1, HL - 1):
                        pool_take = False   # tail stages: DVE is idle there
                    eng = nc.gpsimd if pool_take else nc.vector
                    eng.tensor_tensor(
                        pm[:, 2 * jp : 2 * jp + 2, :],
                        e[:, 2 * jp : 2 * jp + 2, :],
                        mt[:, 4 * g + 2 * jp : 4 * g + 2 * jp + 2, :], MUL,
                    )
                return pm

            def back_av(qc, h, g, pm):
                for j in range(4):
                    kt = 4 * g + j
                    nc.tensor.matmul(
                        ps_av[(qc, h)], v_sb[:, kt, h, :], pm[:, j, :],
                        start=(kt == 0), stop=(kt == KT - 1),
                    )
                if g == 3:
                    nc.scalar.activation(ovt_sb[qc][:, h, :], ps_av[(qc, h)], Copy)

            def emit_norm(qc, hs):
                """1/norm (fp16). Head pairs (2k, 2k+1) are moved by
                contraction-1 PE matmuls to partitions 0 and 64 of one PSUM
                bank, so a single DVE reciprocal (free-size 512, garbage in
                rows 1..63 unused) covers both heads instead of processing
                them serially along the free dim."""
                ovt = ovt_sb[qc]
                for k in range(0, len(hs), 2):
                    pair = hs[k : k + 2]
                    hp = pair[0] // 2
                    psN = psS.tile(
                        [128, 2, QC_W], f32, tag="s2", name=f"psn{qc}_{pair[0]}"
                    )
                    for h in pair:
                        off = (h % 2) * HD
                        nc.tensor.matmul(
                            psN[off : off + 1, 0, :], ones64[HD : HD + 1, 0:1],
                            ovt[HD : HD + 1, h, :], start=True, stop=True,
                        )
                    lo = (pair[0] % 2) * HD
                    hi = (pair[-1] % 2) * HD + 1
                    with nc.allow_low_precision(reason="1/norm fp16; norm ~1e3"):
                        nc.vector.reciprocal(
                            iv_t[qc][lo:hi, hp, :], psN[lo:hi, 0, :]
                        )

            def emit_invmul(qc, h, eng=None):
                # broadcast 1/norm across 64 partitions via PE, land fp16 in
                # SBUF, scale the head output in place on GpSimd (or DVE in
                # the tail where it's idle)
                ps_r = psS.tile([128, 2, QC_W], f32, tag="s2", name=f"psr{qc}_{h}")
                off = (h % 2) * HD
                nc.tensor.matmul(
                    ps_r[0:HD, 0, :], ones64[off : off + 1, 0:HD],
                    iv_t[qc][off : off + 1, h // 2, :], start=True, stop=True,
                )
                rv = rvp.tile([HD, QC_W], f16, tag="rv")
                nc.scalar.activation(rv, ps_r[0:HD, 0, :], Copy)
                (eng or nc.gpsimd).tensor_tensor(
                    ovt_sb[qc][0:HD, h, :], ovt_sb[qc][0:HD, h, :], rv, MUL
                )

            def emit_o_tile(qc, qtl):
                ovn = ovt_sb[qc][0:HD]
                qt_g = qc * (QC_W // 128) + qtl
                ps_o = psS.tile([128, 2, D], f32, tag="s2", name=f"pso{qc}_{qtl}")
                for h in range(HL):
                    nc.tensor.matmul(
                        ps_o[:, 0, :], ovn[:, h, qtl * 128 : (qtl + 1) * 128],
                        wo2_sb[:, h, :], start=(h == 0), stop=(h == HL - 1),
                    )
                fin = finp.tile([128, D], f32, tag="fin")
                nc.scalar.activation(fin, ps_o[:, 0, :], Copy)
                nc.sync.dma_start(
                    out=OUT[qt_g * 128 : (qt_g + 1) * 128, :], in_=fin
                )

            LAG = 3
            queue = []
            push_ctr = [0]

            # side-emissions keyed by completed-push index. Push order:
            # proj chunks c=0..3 interleave (0,0,c),(0,1,c) -> pushes 0..7;
            # then qc0 h2..3 -> 8..15; qc1 h0..3 -> 16..31; qc2 -> 32..47;
            # qc3 -> 48..63. back(i) drains at push i+LAG.
            extras = {}

            def _extra(i, fn):
                extras.setdefault(i, []).append(fn)

            _extra(14, lambda: emit_mt_dma(2))
            _extra(35, lambda: emit_mt_dma(3))
            # back_av (which emits the ovt copies) runs ~4 pushes behind the
            # front: every norm must be EMITTED after its heads' ovt copies
            # (Tile deps follow emission order), and >=1 push before its
            # first invmul so the PE broadcast doesn't head-of-line block.
            _extra(22, lambda: emit_norm(0, [0, 1, 2, 3]))
            for k in range(4):
                _extra(23 + k, lambda k=k: emit_invmul(0, k))
            for k in range(4):
                _extra(27 + 2 * k, lambda k=k: emit_o_tile(0, k))
            _extra(38, lambda: emit_norm(1, [0, 1, 2, 3]))
            for k in range(4):
                _extra(39 + k, lambda k=k: emit_invmul(1, k))
            for k in range(4):
                _extra(43 + 2 * k, lambda k=k: emit_o_tile(1, k))
            _extra(54, lambda: emit_norm(2, [0, 1, 2, 3]))
            for k in range(4):
                _extra(55 + k, lambda k=k: emit_invmul(2, k))
            _extra(59, lambda: emit_o_tile(2, 0))
            _extra(61, lambda: emit_o_tile(2, 1))
            _extra(62, lambda: emit_o_tile(2, 2))
            _extra(63, lambda: emit_o_tile(2, 3))
            _extra(58, lambda: emit_norm(3, [0]))
            _extra(61, lambda: emit_invmul(3, 0))
            _extra(62, lambda: emit_norm(3, [1]))
            _extra(63, lambda: emit_invmul(3, 1))

            av_queue = []
            # during the PE-bound projection window the AV matmuls of early
            # stages are deferred (deep av queue); the attention window (where
            # PE has slack) absorbs them at 2 per push
            av_lag = [99]

            def push(qc, h, g):
                queue.append((qc, h, g, front(qc, h, g)))
                if len(queue) > LAG:
                    qh = queue.pop(0)
                    av_queue.append((qh[0], qh[1], qh[2], back_elem(*qh)))
                burst = 2
                while len(av_queue) > av_lag[0] and burst:
                    back_av(*av_queue.pop(0))
                    burst -= 1
                for fn in extras.get(push_ctr[0], ()):
                    fn()
                push_ctr[0] += 1

            def drain():
                while queue:
                    qh = queue.pop(0)
                    av_queue.append((qh[0], qh[1], qh[2], back_elem(*qh)))
                    while len(av_queue) > 1:
                        back_av(*av_queue.pop(0))
                while av_queue:
                    back_av(*av_queue.pop(0))

            # ---------------- projection phase (+ early attention stages) ----------
            _proj_ctx = ExitStack()
            if True:
                projw = _proj_ctx.enter_context(tc.tile_pool(name="projw", bufs=1))
                xtp = _proj_ctx.enter_context(tc.tile_pool(name="xtp", bufs=2))
                # DMA order: the first K accumulation steps need only the
                # first halves of wxk and xt chunk 0 - split those DMAs so
                # the PE starts earlier
                wxk_sb = projw.tile([128, 8, DL], f16)
                wxk_r = WXK.rearrange("(t p) n -> p t n", p=128)
                nc.sync.dma_start(out=wxk_sb[:, 0:4, :], in_=wxk_r[:, 0:4, :])
                wxq_sb = projw.tile([128, 8, DL], f16)
                wv_sb = projw.tile([128, 4, DL], f16)

                xt_r = XT.rearrange("(t p) s -> p t s", p=128)
                for c in range(4):
                    csl = slice(c * 512, (c + 1) * 512)
                    xt_c = xtp.tile([128, 8, 512], f16, tag="xt")
                    if c == 0:
                        nc.sync.dma_start(out=xt_c[:, 0:4, :], in_=xt_r[:, 0:4, csl])
                        nc.sync.dma_start(out=wxk_sb[:, 4:8, :], in_=wxk_r[:, 4:8, :])
                        nc.sync.dma_start(out=xt_c[:, 4:8, :], in_=xt_r[:, 4:8, csl])
                    else:
                        nc.sync.dma_start(out=xt_c, in_=xt_r[:, :, csl])
                    if c == 0:
                        nc.sync.dma_start(out=wxq_sb, in_=WXQ.rearrange("(t p) n -> p t n", p=128))
                        nc.sync.dma_start(out=wv_sb, in_=WV.rearrange("(t p) n -> p t n", p=128))
                        emit_mt_dma(0)
                    elif c == 1:
                        emit_mt_dma(1)
                        nc.sync.dma_start(
                            out=wo2_sb, in_=WO.rearrange("(h d) n -> d h n", d=HD)
                        )

                    # K^T and Q^T local-head chunks: one [128,2,512] pair each
                    for w_sb, dst in (
                        (wxk_sb, kt_sb[:, :, csl]),
                        (wxq_sb, qt_sb[:, :, csl]),
                    ):
                        ps = psS.tile([128, 2, 512], f32, tag="s2")
                        for j in range(2):
                            for t in range(8):
                                nc.tensor.matmul(
                                    ps[:, j, :], w_sb[:, t, j * 128 : (j + 1) * 128],
                                    xt_c[:, t, :], start=(t == 0), stop=(t == 7),
                                )
                            nc.scalar.activation(dst[:, j, :], ps[:, j, :], Copy)

                    for sp in range(2):         # st pairs
                        ps = psS.tile([128, 2, 512], f32, tag="s2")
                        for j in range(2):
                            st = 2 * sp + j
                            sidx = c * 4 + st
                            for dt in range(4):
                                nc.tensor.matmul(
                                    ps[:, j, 0:DL],
                                    xt_c[:, 4 + dt, st * 128 : (st + 1) * 128],
                                    wv_sb[:, dt, :], start=(dt == 0), stop=(dt == 3),
                                )
                            nc.scalar.activation(
                                v_sb[:, sidx, :, 0:HD],
                                ps[:, j, 0:DL].rearrange("p (h d) -> p h d", h=HL),
                                Copy,
                            )

                    # stage (qc, h, g) needs K/V chunks <= g and Q chunk qc:
                    # start qc0's h0/h1 early
                    push(0, 0, c)
                    push(0, 1, c)
            _proj_ctx.close()
            av_lag[0] = 3

            # ---------------- attention phase ----------------
            for qc in range(N_QC):
                for h in range(HL):
                    if qc == 0 and h < 2:
                        continue    # emitted during projections
                    for g in range(4):
                        push(qc, h, g)
            drain()
            emit_norm(3, [2])
            emit_invmul(3, 2, eng=nc.vector)
            emit_norm(3, [3])
            emit_invmul(3, 3, eng=nc.vector)
            for qtl in range(QC_W // 128):
                emit_o_tile(3, qtl)

    nc.compile()
    return nc


def _get_prog():
    global _PROG
    if _PROG is None:
        _PROG = _build_program()
    return _PROG


def _make_in_maps(inputs):
    f = lambda k: np.asarray(inputs[k], dtype=np.float32)
    gene, expr, M = f("gene_emb"), f("expr_emb"), f("M")
    W_fused = f("W_fused")
    W_Q, W_K, W_V, W_O = f("W_Q"), f("W_K"), f("W_V"), f("W_O")

    scale = np.float32(HD ** -0.5)
    WXQ = (W_fused @ W_Q) * scale
    WXK = W_fused @ W_K

    in_maps = []
    for c in range(8):
        b, hh = c // 2, c % 2
        dsl = slice(hh * DL, (hh + 1) * DL)
        xt = np.concatenate([gene[b], expr[b]], axis=1).T  # [1024, 2048]
        mt = M[b].T                                        # [2048, 2048]
        in_maps.append(
            dict(
                XT=np.ascontiguousarray(xt, dtype=np.float16),
                MT=np.ascontiguousarray(mt, dtype=np.float16),
                WXQ=np.ascontiguousarray(WXQ[:, dsl], dtype=np.float16),
                WXK=np.ascontiguousarray(WXK[:, dsl], dtype=np.float16),
                WV=np.ascontiguousarray(W_V[:, dsl], dtype=np.float16),
                WO=np.ascontiguousarray(W_O[dsl, :], dtype=np.float16),
            )
        )
    return in_maps


def kernel(**inputs) -> np.ndarray:
    from concourse.bass_utils import run_bass_kernel_spmd

    nc = _get_prog()
    in_maps = _make_in_maps(inputs)
    res = run_bass_kernel_spmd(nc, in_maps, core_ids=list(range(8)))

    b_O = np.asarray(inputs["b_O"], dtype=np.float32)
    out = np.empty((B, S, D), dtype=np.float32)
    for b in range(B):
        out[b] = res.results[2 * b]["OUT"] + res.results[2 * b + 1]["OUT"] + b_O[None, :]
    return out


# revision 136
# speedup vs baseline: 1.0048x; 1.0010x over previous
"""Trainium2 Bass kernel for NewExpressionAttentionLayer (sparse gated attention).

Math (per batch b):
  X = concat(gene, expr); Q = X @ (W_fused W_Q scale); K = X @ (W_fused W_K)
  V = expr @ W_V                      (weight folding done on host, fp32)
  t = (Q K^T) * M                     (M = gate)
  p = exp(t)                          (softmax without max-subtraction; |t| <~ 6)
  pm = p * M
  A_bar = pm / sum_k(pm)              (softmax Z cancels; EPS is O(1e-8) rel -> dropped)
  out = (A_bar @ V) @ W_O + b_O       (b_O added on host)

Sharding: 8 cores = 4 batches x 2 head-halves (tensor parallel on heads).
Each core projects Q/K/V for its 4 heads over all 2048 positions and runs
attention for all queries; the host sums the two partial W_O projections.
Head-splitting halves the projection matmul work vs query-splitting (K/V
would be duplicated across the pair) at the cost of shipping the full
[S,S] gate to each core.

Device layout is feature-major: activations [feat, seq] so the PE (contracting
along partitions) needs no on-device transposes. Scores are computed transposed
(scoresT[k, q] = K^T_h.T @ Q^T_h) into fp32 PSUM pair-tiles [128,2,512].
Everything off the PE path is fp16 (2-byte dtype gets DVE 2x mode; fp16
matmuls run 1 row/cycle at any free size).

The attention inner loop is software-pipelined over stages (qc, h, g) where
qc = query chunk of 512, h = local head, g = 4 kt tiles. Front half emits
scores + gate-mul u = scoresT*M + exp; back half (LAG stages later) emits
pm = e*M + 4 AV accumulate matmuls. The gate-mul must read fp32 PSUM, which
only DVE can touch: a tunable fraction is "assisted" (Activation copies the
pair to fp16 SBUF, DVE multiplies in 2x mode) and pm = e*M splits between
DVE (2x) and GpSimd. Normalization, inverse-scale and the W_O projection are
scheduled as side-emissions inside the stage stream so nothing serializes at
chunk boundaries except the final tail.
"""

import sys

sys.path.insert(0, "/opt/trn_rl_repo")

import numpy as np

B, S, D = 4, 2048, 512
H, HD = 8, 64
HL = H // 2          # local heads per core
DL = HL * HD         # 256 local head dims
KT = S // 128        # 16 k partition tiles
QC_W = 512           # query columns per score matmul (PSUM bank limit)
N_QC = S // QC_W     # 4

_PROG = None


def _build_program():
    from contextlib import ExitStack

    from concourse import bacc, mybir
    import concourse.tile as tile

    f32 = mybir.dt.float32
    f16 = mybir.dt.float16
    Exp = mybir.ActivationFunctionType.Exp
    Copy = mybir.ActivationFunctionType.Copy
    MUL = mybir.AluOpType.mult

    nc = bacc.Bacc("TRN2", target_bir_lowering=False, debug=False, num_devices=8)

    XT = nc.dram_tensor("XT", [2 * D, S], f16, kind="ExternalInput").ap()
    MT = nc.dram_tensor("MT", [S, S], f16, kind="ExternalInput").ap()
    WXQ = nc.dram_tensor("WXQ", [2 * D, DL], f16, kind="ExternalInput").ap()
    WXK = nc.dram_tensor("WXK", [2 * D, DL], f16, kind="ExternalInput").ap()
    WV = nc.dram_tensor("WV", [D, DL], f16, kind="ExternalInput").ap()
    WO = nc.dram_tensor("WO", [DL, D], f16, kind="ExternalInput").ap()
    OUT = nc.dram_tensor("OUT", [S, D], f32, kind="ExternalOutput").ap()

    with tile.TileContext(nc) as tc, ExitStack() as _ctx:
            _pool = lambda *a, **k: _ctx.enter_context(tc.tile_pool(*a, **k))
            misc = _pool(name="misc", bufs=1)
            kqv = _pool(name="kqv", bufs=1)
            mtp = _pool(name="mtp", bufs=2)
            up = _pool(name="up", bufs=3)
            ep = _pool(name="ep", bufs=5)
            pmp = _pool(name="pmp", bufs=7)
            ovtp = _pool(name="ovtp", bufs=2)
            ivtp = _pool(name="ivtp", bufs=2)
            rvp = _pool(name="rvp", bufs=2)
            s16p = _pool(name="s16p", bufs=3)
            finp = _pool(name="finp", bufs=2)
            psS = _pool(name="psS", bufs=3, space="PSUM")   # [128,2,512] pairs
            psA = _pool(name="psA", bufs=2, space="PSUM")   # av accumulators

            ones64 = misc.tile([128, 64], f16)
            nc.vector.memset(ones64, 1.0)
            wo2_sb = misc.tile([64, HL, D], f16)

            kt_sb = kqv.tile([128, 2, S], f16)      # K^T [d_local, s]
            qt_sb = kqv.tile([128, 2, S], f16)      # Q^T [d_local, q]
            v_sb = kqv.tile([128, KT, HL, HD + 1], f16)  # V + ones col
            nc.vector.memset(v_sb[:, :, :, HD : HD + 1], 1.0)

            mt_r = MT.rearrange("(t p) q -> p t q", p=128)
            mt_sb = [None] * N_QC

            def emit_mt_dma(qc):
                mt_c = mtp.tile([128, KT, QC_W], f16, tag="mt", name=f"mt{qc}")
                mt_sb[qc] = mt_c
                for q4 in range(4):
                    nc.sync.dma_start(
                        out=mt_c[:, q4 * 4 : (q4 + 1) * 4, :],
                        in_=mt_r[:, q4 * 4 : (q4 + 1) * 4, qc * QC_W : (qc + 1) * QC_W],
                    )

            # ------------- attention stage emitters (software pipelined) -------------
            ovt_sb = [None] * N_QC
            iv_t = [None] * N_QC
            ps_av = {}

            def front(qc, h, g):
                qsl = slice(qc * QC_W, (qc + 1) * QC_W)
                mt = mt_sb[qc]
                hoff = (h % 2) * 64
                ht = h // 2
                if (h, g) == (0, 0):
                    ovt_sb[qc] = ovtp.tile(
                        [HD + 1, HL, QC_W], f16, tag="ovt", name=f"ovt{qc}"
                    )
                    iv_t[qc] = ivtp.tile(
                        [HD + 1, HL // 2, QC_W], f16, tag="ivt", name=f"ivt{qc}"
                    )
                if g == 0:
                    ps_av[(qc, h)] = psA.tile(
                        [HD + 1, QC_W], f32, tag="a", name=f"psav{qc}_{h}"
                    )
                u = up.tile([128, 4, QC_W], f16, tag="u")
                for jp in range(2):
                    ps_s = psS.tile([128, 2, QC_W], f32, tag="s2", name=f"pss{h}_{g}_{jp}")
                    for j2 in range(2):
                        kt = 4 * g + 2 * jp + j2
                        nc.tensor.matmul(
                            ps_s[:, j2, :],
                            kt_sb[hoff : hoff + 64, ht, kt * 128 : (kt + 1) * 128],
                            qt_sb[hoff : hoff + 64, ht, qsl],
                            start=True, stop=True,
                        )
                    # gate-multiply reads fp32 PSUM: GpSimd can't touch PSUM,
                    # so this is DVE-only (the dominant DVE cost). For a
                    # fraction of pairs, Activation (which has slack) copies
                    # the pair to fp16 SBUF so DVE runs it in 2x mode.
                    p = pair_ctr[0]
                    pair_ctr[0] += 1
                    msl = mt[:, 4 * g + 2 * jp : 4 * g + 2 * jp + 2, :]
                    usl = u[:, 2 * jp : 2 * jp + 2, :]
                    if p % 2 == 0 and ((p // 2) * MUL1_ASSIST) % 64 < MUL1_ASSIST:
                        s16 = s16p.tile([128, 2, QC_W], f16, tag="s16")
                        nc.scalar.activation(s16, ps_s, Copy)
                        # once in fp16 SBUF the multiply can also run on
                        # GpSimd - send a few there to shave the DVE wall
                        a = asst_ctr[0]
                        asst_ctr[0] += 1
                        eng = nc.vector
                        eng.tensor_tensor(usl, s16, msl, MUL)
                    else:
                        nc.vector.tensor_tensor(usl, ps_s, msl, MUL)
                e = ep.tile([128, 4, QC_W], f16, tag="e")
                nc.scalar.activation(e, u, Exp)
                return e

            mul2_ctr = [0]
            pair_ctr = [0]
            asst_ctr = [0]
            MUL2_POOL = 66   # of 128 fp16 halves, rest on DVE (2x mode)
            MUL1_ASSIST = 24  # of 128 gate-mul pairs get the Act-assisted path

            def back_elem(qc, h, g, e):
                mt = mt_sb[qc]
                pm = pmp.tile([128, 4, QC_W], f16, tag="pm")
                # pm = e * M in all-SBUF fp16 halves, split DVE (2x mode) /
                # GpSimd to balance (Pool is ~3.6x slower on fp16)
                for jp in range(2):
                    i = mul2_ctr[0]
                    mul2_ctr[0] += 1
                    pool_take = (i % 2) == 1 or (i % 32) == 0
                    if (qc, h) == (N_QC - 1, HL - 1):
                        pool_take = False   # tail stages: DVE is idle there
                    eng = nc.gpsimd if pool_take else nc.vector
                    eng.tensor_tensor(
                        pm[:, 2 * jp : 2 * jp + 2, :],
                        e[:, 2 * jp : 2 * jp + 2, :],
                        mt[:, 4 * g + 2 * jp : 4 * g + 2 * jp + 2, :], MUL,
                    )
                return pm

            def back_av(qc, h, g, pm):
                for j in range(4):
                    kt = 4 * g + j
                    nc.tensor.matmul(
                        ps_av[(qc, h)], v_sb[:, kt, h, :], pm[:, j, :],
                        start=(kt == 0), stop=(kt == KT - 1),
                    )
                if g == 3:
                    nc.scalar.activation(ovt_sb[qc][:, h, :], ps_av[(qc, h)], Copy)

            def emit_norm(qc, hs):
                """1/norm (fp16). Head pairs (2k, 2k+1) are moved by
                contraction-1 PE matmuls to partitions 0 and 64 of one PSUM
                bank, so a single DVE reciprocal (free-size 512, garbage in
                rows 1..63 unused) covers both heads instead of processing
                them serially along the free dim."""
                ovt = ovt_sb[qc]
                for k in range(0, len(hs), 2):
                    pair = hs[k : k + 2]
                    hp = pair[0] // 2
                    psN = psS.tile(
                        [128, 2, QC_W], f32, tag="s2", name=f"psn{qc}_{pair[0]}"
                    )
                    for h in pair:
                        off = (h % 2) * HD
                        nc.tensor.matmul(
                            psN[off : off + 1, 0, :], ones64[HD : HD + 1, 0:1],
                            ovt[HD : HD + 1, h, :], start=True, stop=True,
                        )
                    lo = (pair[0] % 2) * HD
                    hi = (pair[-1] % 2) * HD + 1
                    with nc.allow_low_precision(reason="1/norm fp16; norm ~1e3"):
                        nc.vector.reciprocal(
                            iv_t[qc][lo:hi, hp, :], psN[lo:hi, 0, :]
                        )

            def emit_invmul(qc, h, eng=None):
                # broadcast 1/norm across 64 partitions via PE, land fp16 in
                # SBUF, scale the head output in place on GpSimd (or DVE in
                # the tail where it's idle)
                ps_r = psS.tile([128, 2, QC_W], f32, tag="s2", name=f"psr{qc}_{h}")
                off = (h % 2) * HD
                nc.tensor.matmul(
                    ps_r[0:HD, 0, :], ones64[off : off + 1, 0:HD],
                    iv_t[qc][off : off + 1, h // 2, :], start=True, stop=True,
                )
                rv = rvp.tile([HD, QC_W], f16, tag="rv")
                nc.scalar.activation(rv, ps_r[0:HD, 0, :], Copy)
                (eng or nc.gpsimd).tensor_tensor(
                    ovt_sb[qc][0:HD, h, :], ovt_sb[qc][0:HD, h, :], rv, MUL
                )

            def emit_o_tile(qc, qtl):
                ovn = ovt_sb[qc][0:HD]
                qt_g = qc * (QC_W // 128) + qtl
                ps_o = psS.tile([128, 2, D], f32, tag="s2", name=f"pso{qc}_{qtl}")
                for h in range(HL):
                    nc.tensor.matmul(
                        ps_o[:, 0, :], ovn[:, h, qtl * 128 : (qtl + 1) * 128],
                        wo2_sb[:, h, :], start=(h == 0), stop=(h == HL - 1),
                    )
                fin = finp.tile([128, D], f32, tag="fin")
                nc.scalar.activation(fin, ps_o[:, 0, :], Copy)
                nc.sync.dma_start(
                    out=OUT[qt_g * 128 : (qt_g + 1) * 128, :], in_=fin
                )

            LAG = 3
            queue = []
            push_ctr = [0]

            # side-emissions keyed by completed-push index. Push order:
            # proj chunks c=0..3 interleave (0,0,c),(0,1,c) -> pushes 0..7;
            # then qc0 h2..3 -> 8..15; qc1 h0..3 -> 16..31; qc2 -> 32..47;
            # qc3 -> 48..63. back(i) drains at push i+LAG.
            extras = {}

            def _extra(i, fn):
                extras.setdefault(i, []).append(fn)

            _extra(14, lambda: emit_mt_dma(2))
            _extra(35, lambda: emit_mt_dma(3))
            # back_av (which emits the ovt copies) runs ~4 pushes behind the
            # front: every norm must be EMITTED after its heads' ovt copies
            # (Tile deps follow emission order), and >=1 push before its
            # first invmul so the PE broadcast doesn't head-of-line block.
            _extra(22, lambda: emit_norm(0, [0, 1, 2, 3]))
            for k in range(4):
                _extra(23 + k, lambda k=k: emit_invmul(0, k))
            for k in range(4):
                _extra(27 + 2 * k, lambda k=k: emit_o_tile(0, k))
            _extra(38, lambda: emit_norm(1, [0, 1, 2, 3]))
            for k in range(4):
                _extra(39 + k, lambda k=k: emit_invmul(1, k))
            for k in range(4):
                _extra(43 + 2 * k, lambda k=k: emit_o_tile(1, k))
            _extra(54, lambda: emit_norm(2, [0, 1, 2, 3]))
            for k in range(4):
                _extra(55 + k, lambda k=k: emit_invmul(2, k))
            _extra(59, lambda: emit_o_tile(2, 0))
            _extra(61, lambda: emit_o_tile(2, 1))
            _extra(62, lambda: emit_o_tile(2, 2))
            _extra(63, lambda: emit_o_tile(2, 3))
            _extra(58, lambda: emit_norm(3, [0]))
            _extra(61, lambda: emit_invmul(3, 0))
            _extra(62, lambda: emit_norm(3, [1]))
            _extra(63, lambda: emit_invmul(3, 1))

            av_queue = []
            # during the PE-bound projection window the AV matmuls of early
            # stages are deferred (deep av queue); the attention window (where
            # PE has slack) absorbs them at 2 per push
            av_lag = [99]

            def push(qc, h, g):
                queue.append((qc, h, g, front(qc, h, g)))
                if len(queue) > LAG:
                    qh = queue.pop(0)
                    av_queue.append((qh[0], qh[1], qh[2], back_elem(*qh)))
                burst = 2
                while len(av_queue) > av_lag[0] and burst:
                    back_av(*av_queue.pop(0))
                    burst -= 1
                for fn in extras.get(push_ctr[0], ()):
                    fn()
                push_ctr[0] += 1

            def drain():
                while queue:
                    qh = queue.pop(0)
                    av_queue.append((qh[0], qh[1], qh[2], back_elem(*qh)))
                    while len(av_queue) > 1:
                        back_av(*av_queue.pop(0))
                while av_queue:
                    back_av(*av_queue.pop(0))

            # ---------------- projection phase (+ early attention stages) ----------
            _proj_ctx = ExitStack()
            if True:
                projw = _proj_ctx.enter_context(tc.tile_pool(name="projw", bufs=1))
                xtp = _proj_ctx.enter_context(tc.tile_pool(name="xtp", bufs=2))
                # DMA order: the first K accumulation steps need only the
                # first halves of wxk and xt chunk 0 - split those DMAs so
                # the PE starts earlier
                wxk_sb = projw.tile([128, 8, DL], f16)
                wxk_r = WXK.rearrange("(t p) n -> p t n", p=128)
                nc.sync.dma_start(out=wxk_sb[:, 0:4, :], in_=wxk_r[:, 0:4, :])
                wxq_sb = projw.tile([128, 8, DL], f16)
                wv_sb = projw.tile([128, 4, DL], f16)

                xt_r = XT.rearrange("(t p) s -> p t s", p=128)
                for c in range(4):
                    csl = slice(c * 512, (c + 1) * 512)
                    xt_c = xtp.tile([128, 8, 512], f16, tag="xt")
                    if c == 0:
                        nc.sync.dma_start(out=xt_c[:, 0:4, :], in_=xt_r[:, 0:4, csl])
                        nc.sync.dma_start(out=wxk_sb[:, 4:8, :], in_=wxk_r[:, 4:8, :])
                        nc.sync.dma_start(out=xt_c[:, 4:8, :], in_=xt_r[:, 4:8, csl])
                    else:
                        nc.sync.dma_start(out=xt_c, in_=xt_r[:, :, csl])
                    if c == 0:
                        nc.sync.dma_start(out=wxq_sb, in_=WXQ.rearrange("(t p) n -> p t n", p=128))
                        nc.sync.dma_start(out=wv_sb, in_=WV.rearrange("(t p) n -> p t n", p=128))
                        emit_mt_dma(0)
                    elif c == 1:
                        emit_mt_dma(1)
                        nc.sync.dma_start(
                            out=wo2_sb, in_=WO.rearrange("(h d) n -> d h n", d=HD)
                        )

                    # K^T and Q^T local-head chunks: one [128,2,512] pair each
                    for w_sb, dst in (
                        (wxk_sb, kt_sb[:, :, csl]),
                        (wxq_sb, qt_sb[:, :, csl]),
                    ):
                        ps = psS.tile([128, 2, 512], f32, tag="s2")
                        for j in range(2):
                            for t in range(8):
                                nc.tensor.matmul(
                                    ps[:, j, :], w_sb[:, t, j * 128 : (j + 1) * 128],
                                    xt_c[:, t, :], start=(t == 0), stop=(t == 7),
                                )
                            nc.scalar.activation(dst[:, j, :], ps[:, j, :], Copy)

                    for sp in range(2):         # st pairs
                        ps = psS.tile([128, 2, 512], f32, tag="s2")
                        for j in range(2):
                            st = 2 * sp + j
                            sidx = c * 4 + st
                            for dt in range(4):
                                nc.tensor.matmul(
                                    ps[:, j, 0:DL],
                                    xt_c[:, 4 + dt, st * 128 : (st + 1) * 128],
                                    wv_sb[:, dt, :], start=(dt == 0), stop=(dt == 3),
                                )
                            nc.scalar.activation(
                                v_sb[:, sidx, :, 0:HD],
                                ps[:, j, 0:DL].rearrange("p (h d) -> p h d", h=HL),
                                Copy,
                            )

                    # stage (qc, h, g) needs K/V chunks <= g and Q chunk qc:
                    # start qc0's h0/h1 early
                    push(0, 0, c)
                    push(0, 1, c)
            _proj_ctx.close()
            av_lag[0] = 3

            # ---------------- attention phase ----------------
            for qc in range(N_QC):
                for h in range(HL):
                    if qc == 0 and h < 2:
                        continue    # emitted during projections
                    for g in range(4):
                        push(qc, h, g)
            drain()
            emit_norm(3, [2])
            emit_invmul(3, 2, eng=nc.vector)
            emit_norm(3, [3])
            emit_invmul(3, 3, eng=nc.vector)
            for qtl in range(QC_W // 128):
                emit_o_tile(3, qtl)

    nc.compile()
    return nc


def _get_prog():
    global _PROG
    if _PROG is None:
        _PROG = _build_program()
    return _PROG


def _make_in_maps(inputs):
    f = lambda k: np.asarray(inputs[k], dtype=np.float32)
    gene, expr, M = f("gene_emb"), f("expr_emb"), f("M")
    W_fused = f("W_fused")
    W_Q, W_K, W_V, W_O = f("W_Q"), f("W_K"), f("W_V"), f("W_O")

    scale = np.float32(HD ** -0.5)
    WXQ = (W_fused @ W_Q) * scale
    WXK = W_fused @ W_K

    in_maps = []
    for c in range(8):
        b, hh = c // 2, c % 2
        dsl = slice(hh * DL, (hh + 1) * DL)
        xt = np.concatenate([gene[b], expr[b]], axis=1).T  # [1024, 2048]
        mt = M[b].T                                        # [2048, 2048]
        in_maps.append(
            dict(
                XT=np.ascontiguousarray(xt, dtype=np.float16),
                MT=np.ascontiguousarray(mt, dtype=np.float16),
                WXQ=np.ascontiguousarray(WXQ[:, dsl], dtype=np.float16),
                WXK=np.ascontiguousarray(WXK[:, dsl], dtype=np.float16),
                WV=np.ascontiguousarray(W_V[:, dsl], dtype=np.float16),
                WO=np.ascontiguousarray(W_O[dsl, :], dtype=np.float16),
            )
        )
    return in_maps


def kernel(**inputs) -> np.ndarray:
    from concourse.bass_utils import run_bass_kernel_spmd

    nc = _get_prog()
    in_maps = _make_in_maps(inputs)
    res = run_bass_kernel_spmd(nc, in_maps, core_ids=list(range(8)))

    b_O = np.asarray(inputs["b_O"], dtype=np.float32)
    out = np.empty((B, S, D), dtype=np.float32)
    for b in range(B):
        out[b] = res.results[2 * b]["OUT"] + res.results[2 * b + 1]["OUT"] + b_O[None, :]
    return out


# revision 137
# speedup vs baseline: 1.0081x; 1.0033x over previous
"""Trainium2 Bass kernel for NewExpressionAttentionLayer (sparse gated attention).

Math (per batch b):
  X = concat(gene, expr); Q = X @ (W_fused W_Q scale); K = X @ (W_fused W_K)
  V = expr @ W_V                      (weight folding done on host, fp32)
  t = (Q K^T) * M                     (M = gate)
  p = exp(t)                          (softmax without max-subtraction; |t| <~ 6)
  pm = p * M
  A_bar = pm / sum_k(pm)              (softmax Z cancels; EPS is O(1e-8) rel -> dropped)
  out = (A_bar @ V) @ W_O + b_O       (b_O added on host)

Sharding: 8 cores = 4 batches x 2 head-halves (tensor parallel on heads).
Each core projects Q/K/V for its 4 heads over all 2048 positions and runs
attention for all queries; the host sums the two partial W_O projections.
Head-splitting halves the projection matmul work vs query-splitting (K/V
would be duplicated across the pair) at the cost of shipping the full
[S,S] gate to each core.

Device layout is feature-major: activations [feat, seq] so the PE (contracting
along partitions) needs no on-device transposes. Scores are computed transposed
(scoresT[k, q] = K^T_h.T @ Q^T_h) into fp32 PSUM pair-tiles [128,2,512].
Everything off the PE path is fp16 (2-byte dtype gets DVE 2x mode; fp16
matmuls run 1 row/cycle at any free size).

The attention inner loop is software-pipelined over stages (qc, h, g) where
qc = query chunk of 512, h = local head, g = 4 kt tiles. Front half emits
scores + gate-mul u = scoresT*M + exp; back half (LAG stages later) emits
pm = e*M + 4 AV accumulate matmuls. The gate-mul must read fp32 PSUM, which
only DVE can touch: a tunable fraction is "assisted" (Activation copies the
pair to fp16 SBUF, DVE multiplies in 2x mode) and pm = e*M splits between
DVE (2x) and GpSimd. Normalization, inverse-scale and the W_O projection are
scheduled as side-emissions inside the stage stream so nothing serializes at
chunk boundaries except the final tail.
"""

import sys

sys.path.insert(0, "/opt/trn_rl_repo")

import numpy as np

B, S, D = 4, 2048, 512
H, HD = 8, 64
HL = H // 2          # local heads per core
DL = HL * HD         # 256 local head dims
KT = S // 128        # 16 k partition tiles
QC_W = 512           # query columns per score matmul (PSUM bank limit)
N_QC = S // QC_W     # 4

_PROG = None


def _build_program():
    from contextlib import ExitStack

    from concourse import bacc, mybir
    import concourse.tile as tile

    f32 = mybir.dt.float32
    f16 = mybir.dt.float16
    Exp = mybir.ActivationFunctionType.Exp
    Copy = mybir.ActivationFunctionType.Copy
    MUL = mybir.AluOpType.mult

    nc = bacc.Bacc("TRN2", target_bir_lowering=False, debug=False, num_devices=8)

    XT = nc.dram_tensor("XT", [2 * D, S], f16, kind="ExternalInput").ap()
    MT = nc.dram_tensor("MT", [S, S], f16, kind="ExternalInput").ap()
    WXQ = nc.dram_tensor("WXQ", [2 * D, DL], f16, kind="ExternalInput").ap()
    WXK = nc.dram_tensor("WXK", [2 * D, DL], f16, kind="ExternalInput").ap()
    WV = nc.dram_tensor("WV", [D, DL], f16, kind="ExternalInput").ap()
    WO = nc.dram_tensor("WO", [DL, D], f16, kind="ExternalInput").ap()
    OUT = nc.dram_tensor("OUT", [S, D], f32, kind="ExternalOutput").ap()

    with tile.TileContext(nc) as tc, ExitStack() as _ctx:
            _pool = lambda *a, **k: _ctx.enter_context(tc.tile_pool(*a, **k))
            misc = _pool(name="misc", bufs=1)
            kqv = _pool(name="kqv", bufs=1)
            mtp = _pool(name="mtp", bufs=2)
            up = _pool(name="up", bufs=4)
            ep = _pool(name="ep", bufs=5)
            pmp = _pool(name="pmp", bufs=7)
            ovtp = _pool(name="ovtp", bufs=2)
            ivtp = _pool(name="ivtp", bufs=2)
            rvp = _pool(name="rvp", bufs=3)
            s16p = _pool(name="s16p", bufs=3)
            finp = _pool(name="finp", bufs=3)
            psS = _pool(name="psS", bufs=3, space="PSUM")   # [128,2,512] pairs
            psA = _pool(name="psA", bufs=2, space="PSUM")   # av accumulators

            ones64 = misc.tile([128, 64], f16)
            nc.vector.memset(ones64, 1.0)
            wo2_sb = misc.tile([64, HL, D], f16)

            kt_sb = kqv.tile([128, 2, S], f16)      # K^T [d_local, s]
            qt_sb = kqv.tile([128, 2, S], f16)      # Q^T [d_local, q]
            v_sb = kqv.tile([128, KT, HL, HD + 1], f16)  # V + ones col
            nc.vector.memset(v_sb[:, :, :, HD : HD + 1], 1.0)

            mt_r = MT.rearrange("(t p) q -> p t q", p=128)
            mt_sb = [None] * N_QC

            def emit_mt_dma(qc):
                mt_c = mtp.tile([128, KT, QC_W], f16, tag="mt", name=f"mt{qc}")
                mt_sb[qc] = mt_c
                for q4 in range(4):
                    nc.sync.dma_start(
                        out=mt_c[:, q4 * 4 : (q4 + 1) * 4, :],
                        in_=mt_r[:, q4 * 4 : (q4 + 1) * 4, qc * QC_W : (qc + 1) * QC_W],
                    )

            # ------------- attention stage emitters (software pipelined) -------------
            ovt_sb = [None] * N_QC
            iv_t = [None] * N_QC
            ps_av = {}

            def front(qc, h, g):
                qsl = slice(qc * QC_W, (qc + 1) * QC_W)
                mt = mt_sb[qc]
                hoff = (h % 2) * 64
                ht = h // 2
                if (h, g) == (0, 0):
                    ovt_sb[qc] = ovtp.tile(
                        [HD + 1, HL, QC_W], f16, tag="ovt", name=f"ovt{qc}"
                    )
                    iv_t[qc] = ivtp.tile(
                        [HD + 1, HL // 2, QC_W], f16, tag="ivt", name=f"ivt{qc}"
                    )
                if g == 0:
                    ps_av[(qc, h)] = psA.tile(
                        [HD + 1, QC_W], f32, tag="a", name=f"psav{qc}_{h}"
                    )
                u = up.tile([128, 4, QC_W], f16, tag="u")
                for jp in range(2):
                    ps_s = psS.tile([128, 2, QC_W], f32, tag="s2", name=f"pss{h}_{g}_{jp}")
                    for j2 in range(2):
                        kt = 4 * g + 2 * jp + j2
                        nc.tensor.matmul(
                            ps_s[:, j2, :],
                            kt_sb[hoff : hoff + 64, ht, kt * 128 : (kt + 1) * 128],
                            qt_sb[hoff : hoff + 64, ht, qsl],
                            start=True, stop=True,
                        )
                    # gate-multiply reads fp32 PSUM: GpSimd can't touch PSUM,
                    # so this is DVE-only (the dominant DVE cost). For a
                    # fraction of pairs, Activation (which has slack) copies
                    # the pair to fp16 SBUF so DVE runs it in 2x mode.
                    p = pair_ctr[0]
                    pair_ctr[0] += 1
                    msl = mt[:, 4 * g + 2 * jp : 4 * g + 2 * jp + 2, :]
                    usl = u[:, 2 * jp : 2 * jp + 2, :]
                    if p % 2 == 0 and ((p // 2) * MUL1_ASSIST) % 64 < MUL1_ASSIST:
                        s16 = s16p.tile([128, 2, QC_W], f16, tag="s16")
                        nc.scalar.activation(s16, ps_s, Copy)
                        # once in fp16 SBUF the multiply can also run on
                        # GpSimd - send a few there to shave the DVE wall
                        a = asst_ctr[0]
                        asst_ctr[0] += 1
                        eng = nc.vector
                        eng.tensor_tensor(usl, s16, msl, MUL)
                    else:
                        nc.vector.tensor_tensor(usl, ps_s, msl, MUL)
                e = ep.tile([128, 4, QC_W], f16, tag="e")
                nc.scalar.activation(e, u, Exp)
                return e

            mul2_ctr = [0]
            pair_ctr = [0]
            asst_ctr = [0]
            MUL2_POOL = 66   # of 128 fp16 halves, rest on DVE (2x mode)
            MUL1_ASSIST = 24  # of 128 gate-mul pairs get the Act-assisted path

            def back_elem(qc, h, g, e):
                mt = mt_sb[qc]
                pm = pmp.tile([128, 4, QC_W], f16, tag="pm")
                # pm = e * M in all-SBUF fp16 halves, split DVE (2x mode) /
                # GpSimd to balance (Pool is ~3.6x slower on fp16)
                for jp in range(2):
                    i = mul2_ctr[0]
                    mul2_ctr[0] += 1
                    pool_take = (i % 2) == 1 or (i % 32) == 0
                    if (qc, h) == (N_QC - 1, HL - 1):
                        pool_take = False   # tail stages: DVE is idle there
                    eng = nc.gpsimd if pool_take else nc.vector
                    eng.tensor_tensor(
                        pm[:, 2 * jp : 2 * jp + 2, :],
                        e[:, 2 * jp : 2 * jp + 2, :],
                        mt[:, 4 * g + 2 * jp : 4 * g + 2 * jp + 2, :], MUL,
                    )
                return pm

            def back_av(qc, h, g, pm):
                for j in range(4):
                    kt = 4 * g + j
                    nc.tensor.matmul(
                        ps_av[(qc, h)], v_sb[:, kt, h, :], pm[:, j, :],
                        start=(kt == 0), stop=(kt == KT - 1),
                    )
                if g == 3:
                    nc.scalar.activation(ovt_sb[qc][:, h, :], ps_av[(qc, h)], Copy)

            def emit_norm(qc, hs):
                """1/norm (fp16). Head pairs (2k, 2k+1) are moved by
                contraction-1 PE matmuls to partitions 0 and 64 of one PSUM
                bank, so a single DVE reciprocal (free-size 512, garbage in
                rows 1..63 unused) covers both heads instead of processing
                them serially along the free dim."""
                ovt = ovt_sb[qc]
                for k in range(0, len(hs), 2):
                    pair = hs[k : k + 2]
                    hp = pair[0] // 2
                    psN = psS.tile(
                        [128, 2, QC_W], f32, tag="s2", name=f"psn{qc}_{pair[0]}"
                    )
                    for h in pair:
                        off = (h % 2) * HD
                        nc.tensor.matmul(
                            psN[off : off + 1, 0, :], ones64[HD : HD + 1, 0:1],
                            ovt[HD : HD + 1, h, :], start=True, stop=True,
                        )
                    lo = (pair[0] % 2) * HD
                    hi = (pair[-1] % 2) * HD + 1
                    with nc.allow_low_precision(reason="1/norm fp16; norm ~1e3"):
                        nc.vector.reciprocal(
                            iv_t[qc][lo:hi, hp, :], psN[lo:hi, 0, :]
                        )

            def emit_invmul(qc, h, eng=None):
                # broadcast 1/norm across 64 partitions via PE, land fp16 in
                # SBUF, scale the head output in place on GpSimd (or DVE in
                # the tail where it's idle)
                ps_r = psS.tile([128, 2, QC_W], f32, tag="s2", name=f"psr{qc}_{h}")
                off = (h % 2) * HD
                nc.tensor.matmul(
                    ps_r[0:HD, 0, :], ones64[off : off + 1, 0:HD],
                    iv_t[qc][off : off + 1, h // 2, :], start=True, stop=True,
                )
                rv = rvp.tile([HD, QC_W], f16, tag="rv")
                nc.scalar.activation(rv, ps_r[0:HD, 0, :], Copy)
                (eng or nc.gpsimd).tensor_tensor(
                    ovt_sb[qc][0:HD, h, :], ovt_sb[qc][0:HD, h, :], rv, MUL
                )

            def emit_o_tile(qc, qtl):
                ovn = ovt_sb[qc][0:HD]
                qt_g = qc * (QC_W // 128) + qtl
                ps_o = psS.tile([128, 2, D], f32, tag="s2", name=f"pso{qc}_{qtl}")
                for h in range(HL):
                    nc.tensor.matmul(
                        ps_o[:, 0, :], ovn[:, h, qtl * 128 : (qtl + 1) * 128],
                        wo2_sb[:, h, :], start=(h == 0), stop=(h == HL - 1),
                    )
                fin = finp.tile([128, D], f32, tag="fin")
                nc.scalar.activation(fin, ps_o[:, 0, :], Copy)
                nc.sync.dma_start(
                    out=OUT[qt_g * 128 : (qt_g + 1) * 128, :], in_=fin
                )

            LAG = 3
            queue = []
            push_ctr = [0]

            # side-emissions keyed by completed-push index. Push order:
            # proj chunks c=0..3 interleave (0,0,c),(0,1,c) -> pushes 0..7;
            # then qc0 h2..3 -> 8..15; qc1 h0..3 -> 16..31; qc2 -> 32..47;
            # qc3 -> 48..63. back(i) drains at push i+LAG.
            extras = {}

            def _extra(i, fn):
                extras.setdefault(i, []).append(fn)

            _extra(14, lambda: emit_mt_dma(2))
            _extra(35, lambda: emit_mt_dma(3))
            # back_av (which emits the ovt copies) runs ~4 pushes behind the
            # front: every norm must be EMITTED after its heads' ovt copies
            # (Tile deps follow emission order), and >=1 push before its
            # first invmul so the PE broadcast doesn't head-of-line block.
            _extra(22, lambda: emit_norm(0, [0, 1, 2, 3]))
            for k in range(4):
                _extra(23 + k, lambda k=k: emit_invmul(0, k))
            for k in range(4):
                _extra(27 + 2 * k, lambda k=k: emit_o_tile(0, k))
            _extra(38, lambda: emit_norm(1, [0, 1, 2, 3]))
            for k in range(4):
                _extra(39 + k, lambda k=k: emit_invmul(1, k))
            for k in range(4):
                _extra(43 + 2 * k, lambda k=k: emit_o_tile(1, k))
            _extra(54, lambda: emit_norm(2, [0, 1, 2, 3]))
            for k in range(4):
                _extra(55 + k, lambda k=k: emit_invmul(2, k))
            _extra(59, lambda: emit_o_tile(2, 0))
            _extra(61, lambda: emit_o_tile(2, 1))
            _extra(62, lambda: emit_o_tile(2, 2))
            _extra(63, lambda: emit_o_tile(2, 3))
            _extra(58, lambda: emit_norm(3, [0]))
            _extra(61, lambda: emit_invmul(3, 0))
            _extra(62, lambda: emit_norm(3, [1]))
            _extra(63, lambda: emit_invmul(3, 1))

            av_queue = []
            # during the PE-bound projection window the AV matmuls of early
            # stages are deferred (deep av queue); the attention window (where
            # PE has slack) absorbs them at 2 per push
            av_lag = [99]

            def push(qc, h, g):
                queue.append((qc, h, g, front(qc, h, g)))
                if len(queue) > LAG:
                    qh = queue.pop(0)
                    av_queue.append((qh[0], qh[1], qh[2], back_elem(*qh)))
                burst = 2
                while len(av_queue) > av_lag[0] and burst:
                    back_av(*av_queue.pop(0))
                    burst -= 1
                for fn in extras.get(push_ctr[0], ()):
                    fn()
                push_ctr[0] += 1

            def drain():
                while queue:
                    qh = queue.pop(0)
                    av_queue.append((qh[0], qh[1], qh[2], back_elem(*qh)))
                    while len(av_queue) > 1:
                        back_av(*av_queue.pop(0))
                while av_queue:
                    back_av(*av_queue.pop(0))

            # ---------------- projection phase (+ early attention stages) ----------
            _proj_ctx = ExitStack()
            if True:
                projw = _proj_ctx.enter_context(tc.tile_pool(name="projw", bufs=1))
                xtp = _proj_ctx.enter_context(tc.tile_pool(name="xtp", bufs=2))
                # DMA order: the first K accumulation steps need only the
                # first halves of wxk and xt chunk 0 - split those DMAs so
                # the PE starts earlier
                wxk_sb = projw.tile([128, 8, DL], f16)
                wxk_r = WXK.rearrange("(t p) n -> p t n", p=128)
                nc.sync.dma_start(out=wxk_sb[:, 0:4, :], in_=wxk_r[:, 0:4, :])
                wxq_sb = projw.tile([128, 8, DL], f16)
                wv_sb = projw.tile([128, 4, DL], f16)

                xt_r = XT.rearrange("(t p) s -> p t s", p=128)
                for c in range(4):
                    csl = slice(c * 512, (c + 1) * 512)
                    xt_c = xtp.tile([128, 8, 512], f16, tag="xt")
                    if c == 0:
                        nc.sync.dma_start(out=xt_c[:, 0:4, :], in_=xt_r[:, 0:4, csl])
                        nc.sync.dma_start(out=wxk_sb[:, 4:8, :], in_=wxk_r[:, 4:8, :])
                        nc.sync.dma_start(out=xt_c[:, 4:8, :], in_=xt_r[:, 4:8, csl])
                    else:
                        nc.sync.dma_start(out=xt_c, in_=xt_r[:, :, csl])
                    if c == 0:
                        nc.sync.dma_start(out=wxq_sb, in_=WXQ.rearrange("(t p) n -> p t n", p=128))
                        nc.sync.dma_start(out=wv_sb, in_=WV.rearrange("(t p) n -> p t n", p=128))
                        emit_mt_dma(0)
                    elif c == 1:
                        emit_mt_dma(1)
                        nc.sync.dma_start(
                            out=wo2_sb, in_=WO.rearrange("(h d) n -> d h n", d=HD)
                        )

                    # K^T and Q^T local-head chunks: one [128,2,512] pair each
                    for w_sb, dst in (
                        (wxk_sb, kt_sb[:, :, csl]),
                        (wxq_sb, qt_sb[:, :, csl]),
                    ):
                        ps = psS.tile([128, 2, 512], f32, tag="s2")
                        for j in range(2):
                            for t in range(8):
                                nc.tensor.matmul(
                                    ps[:, j, :], w_sb[:, t, j * 128 : (j + 1) * 128],
                                    xt_c[:, t, :], start=(t == 0), stop=(t == 7),
                                )
                            nc.scalar.activation(dst[:, j, :], ps[:, j, :], Copy)

                    for sp in range(2):         # st pairs
                        ps = psS.tile([128, 2, 512], f32, tag="s2")
                        for j in range(2):
                            st = 2 * sp + j
                            sidx = c * 4 + st
                            for dt in range(4):
                                nc.tensor.matmul(
                                    ps[:, j, 0:DL],
                                    xt_c[:, 4 + dt, st * 128 : (st + 1) * 128],
                                    wv_sb[:, dt, :], start=(dt == 0), stop=(dt == 3),
                                )
                            nc.scalar.activation(
                                v_sb[:, sidx, :, 0:HD],
                                ps[:, j, 0:DL].rearrange("p (h d) -> p h d", h=HL),
                                Copy,
                            )

                    # stage (qc, h, g) needs K/V chunks <= g and Q chunk qc:
                    # start qc0's h0/h1 early
                    push(0, 0, c)
                    push(0, 1, c)
            _proj_ctx.close()
            av_lag[0] = 3

            # ---------------- attention phase ----------------
            for qc in range(N_QC):
                for h in range(HL):
                    if qc == 0 and h < 2:
                        continue    # emitted during projections
                    for g in range(4):
                        push(qc, h, g)
            drain()
            emit_norm(3, [2])
            emit_invmul(3, 2, eng=nc.vector)
            emit_norm(3, [3])
            emit_invmul(3, 3, eng=nc.vector)
            for qtl in range(QC_W // 128):
                emit_o_tile(3, qtl)

    nc.compile()
    return nc


def _get_prog():
    global _PROG
    if _PROG is None:
        _PROG = _build_program()
    return _PROG


def _make_in_maps(inputs):
    f = lambda k: np.asarray(inputs[k], dtype=np.float32)
    gene, expr, M = f("gene_emb"), f("expr_emb"), f("M")
    W_fused = f("W_fused")
    W_Q, W_K, W_V, W_O = f("W_Q"), f("W_K"), f("W_V"), f("W_O")

    scale = np.float32(HD ** -0.5)
    WXQ = (W_fused @ W_Q) * scale
    WXK = W_fused @ W_K

    in_maps = []
    for c in range(8):
        b, hh = c // 2, c % 2
        dsl = slice(hh * DL, (hh + 1) * DL)
        xt = np.concatenate([gene[b], expr[b]], axis=1).T  # [1024, 2048]
        mt = M[b].T                                        # [2048, 2048]
        in_maps.append(
            dict(
                XT=np.ascontiguousarray(xt, dtype=np.float16),
                MT=np.ascontiguousarray(mt, dtype=np.float16),
                WXQ=np.ascontiguousarray(WXQ[:, dsl], dtype=np.float16),
                WXK=np.ascontiguousarray(WXK[:, dsl], dtype=np.float16),
                WV=np.ascontiguousarray(W_V[:, dsl], dtype=np.float16),
                WO=np.ascontiguousarray(W_O[dsl, :], dtype=np.float16),
            )
        )
    return in_maps


def kernel(**inputs) -> np.ndarray:
    from concourse.bass_utils import run_bass_kernel_spmd

    nc = _get_prog()
    in_maps = _make_in_maps(inputs)
    res = run_bass_kernel_spmd(nc, in_maps, core_ids=list(range(8)))

    b_O = np.asarray(inputs["b_O"], dtype=np.float32)
    out = np.empty((B, S, D), dtype=np.float32)
    for b in range(B):
        out[b] = res.results[2 * b]["OUT"] + res.results[2 * b + 1]["OUT"] + b_O[None, :]
    return out
